# revision 22
# baseline (speedup 1.0000x reference)
"""Distributed KNN retrieval (Database topk=4) on 8 Trainium2 NeuronCores.

Pipeline (per core, SPMD over 8 cores; corpus sharded along N):
  1. Scan of the core's 50000-column shard in 2048-column chunks:
     fp8e4 DoubleRow matmul (raw queries -- per-query ranking is scale
     invariant) -> PSUM fp32 sims.
  2. ScalarE packs value+index in ONE pass: it copies PSUM fp32 -> bf16
     with a stride-2 write into the HIGH halfwords of a u32 tile whose low
     halfwords are pre-filled with the column iota. The u32 tile read as
     fp32 orders by similarity (low 16 bits are sub-ulp noise) and its low
     bits carry the in-chunk column.
  3. DVE max8 per chunk on packed -> per-core candidate list [128, 200].
     The raw packed lists are DMA'd straight to DRAM -- no device-side
     level-2 / gather / rescore tail at all.
Host merges 8 cores x 200 packed candidates per query, decodes
(value | in-chunk column) + chunk position, exact-rescores the top
candidates in fp32 and takes the global top-4.

The masked range [start, end) is zeroed in the fp8 shard: masked sims are
exactly 0 and never reach the per-core per-chunk top-8 (top sims are
strictly positive); the host additionally filters any candidate falling in
[start, end) or in the shard tail padding before rescoring.
"""

import os

import numpy as np
import ml_dtypes

import concourse.bass as bass
import concourse.bacc as bacc
import concourse.mybir as mybir
import concourse.tile as tile
import concourse.bass_utils as bass_utils

Q, D, N, TOPK = 256, 768, 400000, 4
NCORES = 8
NSHARD = N // NCORES          # 50000
CHUNK = 2048
CHUNK_SH = 11                 # log2(CHUNK)
NCH = (NSHARD + CHUNK - 1) // CHUNK   # 25
NPAD = NCH * CHUNK            # 51200
KT2 = D // 256                # 3 DoubleRow k-tiles (256-deep each)
MT = Q // 128                 # 2 m-tiles
CAND = NCH * 8                # 200 level-1 candidates per core per query
NPK = 4                       # packed-tile ring depth
LASTPAD = 1024                # padded width scanned in the last chunk
EMB_SCALE = 512.0             # fp8 range scaling (ranking invariant)
K_HOST = 64                   # candidates exact-rescored per query on host



_prog_cache = {}


def _install_ntff_hook_shim():
    """Provide antenv.axon_hooks (absent in this image) so that
    run_bass_kernel_spmd(trace=True) can capture NTFF profiles through the
    injected libaxon_pjrt.so. Mirrors trn_agent_boot/trn_boot.py."""
    import sys
    import types
    import ctypes
    import contextlib

    if "antenv.axon_hooks" in sys.modules:
        return
    mod = types.ModuleType("antenv.axon_hooks")
    state = {"hook": None}
    mod.set_axon_ntff_profile_hook = lambda h: state.__setitem__("hook", h)
    mod.get_axon_ntff_profile_hook = lambda: state["hook"]
    sys.modules["antenv.axon_hooks"] = mod

    so_path = "/opt/axon/libaxon_pjrt.so"
    if not os.path.exists(so_path):
        return
    try:
        lib = ctypes.CDLL(so_path)
    except OSError:
        return
    if not hasattr(lib, "axon_start_nrt_profile"):
        return
    lib.axon_start_nrt_profile.argtypes = [ctypes.POINTER(ctypes.c_int64),
                                           ctypes.c_size_t]
    lib.axon_start_nrt_profile.restype = ctypes.c_int64
    lib.axon_stop_nrt_profile.argtypes = [ctypes.c_char_p]
    lib.axon_stop_nrt_profile.restype = ctypes.c_int64

    @contextlib.contextmanager
    def _hook(output_dir, device_ids):
        import jax
        jax.devices()
        if device_ids:
            ids = (ctypes.c_int64 * len(device_ids))(*device_ids)
            rc = lib.axon_start_nrt_profile(ids, len(device_ids))
        else:
            rc = lib.axon_start_nrt_profile(None, 0)
        if rc != 0:
            raise RuntimeError(f"axon_start_nrt_profile rc={rc}")
        try:
            yield
        finally:
            n = lib.axon_stop_nrt_profile(str(output_dir).encode())
            print(f"ntff profile: {n} file(s) written to {output_dir}")

    mod.set_axon_ntff_profile_hook(_hook)


def _build_program():
    nc = bacc.Bacc(None, target_bir_lowering=False, debug=False)
    f8 = mybir.dt.float8e4
    u32 = mybir.dt.uint32
    f32 = mybir.dt.float32
    bf16 = mybir.dt.bfloat16

    # raw queries, fp8, host-laid-out to match the SBUF tile exactly:
    # qt[p, t, i, m] = q8[m, t*256 + i*128 + p] -- one contiguous DMA
    qt_dram = nc.dram_tensor("qT", [128, KT2, 2, Q], f8, kind="ExternalInput")
    # emb shard, fp8, host-packed DoubleRow layout:
    # embL[j, p, (t*2 + i)*CHUNK + n] = emb8[t*256 + i*128 + p, j*CHUNK + n]
    embL = nc.dram_tensor("embL", [NCH, 128, KT2 * 2 * CHUNK], f8,
                          kind="ExternalInput")

    # packed candidate lists, one row of CAND u32 per query (emitted as f32
    # bit patterns; host reinterprets)
    out_cand = nc.dram_tensor("out_cand", [MT, 128, CAND], f32,
                              kind="ExternalOutput")

    DR = mybir.MatmulPerfMode.DoubleRow

    with tile.TileContext(nc) as tc:
        with tc.tile_pool(name="persist", bufs=1) as pp:
            qT = pp.tile([128, KT2, 2, Q], f8, tag="qT")
            vals_all = [pp.tile([128, CAND], f32, tag=f"va{m}", name=f"va{m}")
                        for m in range(MT)]
            # packed ring: low halfwords hold the column iota permanently
            pk = [pp.tile([128, CHUNK], u32, tag=f"pk{i}", name=f"pk{i}")
                  for i in range(NPK)]

            nc.sync.dma_start(qT[:], qt_dram.ap())
            for i in range(NPK):
                nc.gpsimd.iota(pk[i][:], pattern=[[1, CHUNK]], base=0,
                               channel_multiplier=0)

            # ---------- scan shard ----------
            with (
                tc.tile_pool(name="rhs_sb", bufs=6) as rp,
                tc.tile_pool(name="sim_ps", bufs=2, space="PSUM") as sps,
            ):
                # warm the tensor engine to full p-state while the first
                # rhs chunk is still in flight (depends only on qT)
                warm = sps.tile([128, CHUNK], f32, tag="sim")
                for _ in range(6):
                    nc.tensor.matmul(warm[:, :Q], qT[:, 0, :, :128],
                                     qT[:, 0, :, :], start=True, stop=True,
                                     perf_mode=DR)
                for j in range(NCH):
                    w = LASTPAD if j == NCH - 1 else CHUNK
                    rhs = rp.tile([128, KT2, 2, CHUNK], f8, tag="rhs")
                    src = embL.ap()[j].rearrange("p (t i n) -> p t i n",
                                                 t=KT2, i=2)
                    if j == 0:
                        # split per k-tile so the first matmul can start
                        # after one third of the chunk has landed
                        for t in range(KT2):
                            nc.sync.dma_start(rhs[:, t, :, :w],
                                              src[:, t, :, :w])
                    else:
                        # one DMA per chunk: a single large transfer keeps
                        # the 16 DMA engines at full rate and only costs one
                        # serialized SP issue per buffer-release
                        nc.sync.dma_start(rhs[:, :, :, :w], src[:, :, :, :w])
                    for m in range(MT):
                        psum = sps.tile([128, CHUNK], f32, tag="sim")
                        for t in range(KT2):
                            for h in range(w // 512):
                                nc.tensor.matmul(
                                    psum[:, h * 512:(h + 1) * 512],
                                    qT[:, t, :, m * 128:(m + 1) * 128],
                                    rhs[:, t, :, h * 512:(h + 1) * 512],
                                    start=(t == 0), stop=(t == KT2 - 1),
                                    perf_mode=DR)
                        packed = pk[(j * MT + m) % NPK]
                        dst = packed[:].bitcast(bf16)
                        nc.scalar.copy(dst[:, 1:2 * w:2], psum[:, :w])
                        nc.vector.max(vals_all[m][:, j * 8:(j + 1) * 8],
                                      packed[:, :w].bitcast(f32))
                        if j == NCH - 1:
                            nc.sync.dma_start(out_cand.ap()[m], vals_all[m][:])

    nc.compile()
    return nc


def _get_program():
    if "nc" not in _prog_cache:
        _prog_cache["nc"] = _build_program()
    return _prog_cache["nc"]


def _prepare_core_inputs(q, emb, start, end):
    """Shard + pack fp8 inputs for each core. Returns list of per-core dicts."""
    f8 = ml_dtypes.float8_e4m3
    embs = emb * np.float32(EMB_SCALE)
    if end > start:
        embs[:, start:end] = 0
    emb8 = np.clip(embs, -240, 240).astype(f8)
    q8 = np.clip(np.asarray(q, dtype=np.float32), -240, 240).astype(f8)
    # qt[p, t, i, m] = q8[m, t*256 + i*128 + p]
    qt = np.ascontiguousarray(
        q8.T.reshape(KT2, 2, 128, Q).transpose(2, 0, 1, 3))
    in_maps = []
    for c in range(NCORES):
        lo = c * NSHARD
        pad = np.zeros((D, NPAD), dtype=f8)
        pad[:, :NSHARD] = emb8[:, lo:lo + NSHARD]
        # embL[j, p, (t*2+i)*CHUNK + n] = pad[t*256 + i*128 + p, j*CHUNK + n]
        embL = np.ascontiguousarray(
            pad.reshape(KT2 * 2, 128, NCH, CHUNK).transpose(2, 1, 0, 3)
        ).reshape(NCH, 128, KT2 * 2 * CHUNK)
        in_maps.append({"qT": qt, "embL": embL})
    return in_maps


def kernel(query, embeddings, start, end):
    q = np.asarray(query, dtype=np.float32)
    emb = np.asarray(embeddings, dtype=np.float32)
    start_i = int(np.asarray(start))
    end_i = int(np.asarray(end))
    assert q.shape == (Q, D) and emb.shape == (D, N)

    nc = _get_program()
    in_maps = _prepare_core_inputs(q, emb, start_i, end_i)

    trace = os.environ.get("KNN_TRACE", "0") == "1"
    if trace:
        _install_ntff_hook_shim()
    res = bass_utils.run_bass_kernel_spmd(
        nc, in_maps, core_ids=list(range(NCORES)), trace=trace)
    if trace:
        _prog_cache["last_exec_time_ns"] = res.exec_time_ns
        _prog_cache["last_results"] = res

    # [8, MT, 128, CAND] u32 packed (bf16 sim | in-chunk column)
    packed = np.stack([r["out_cand"] for r in res.results]).view(np.uint32)

    # decode: approximate value from the high halfword, column from the
    # candidate's list position (chunk) + low bits (in-chunk column)
    vals = (packed & np.uint32(0xFFFF0000)).view(np.float32)
    chunk = (np.arange(CAND, dtype=np.int64) >> 3) << CHUNK_SH
    scol = chunk[None, None, None, :] + (packed & np.uint32(CHUNK - 1))
    gcol = scol + (np.arange(NCORES, dtype=np.int64) * NSHARD)[:, None, None, None]

    # per query: [Q, NCORES*CAND]
    allv = vals.transpose(1, 2, 0, 3).reshape(Q, NCORES * CAND).copy()
    allg = gcol.transpose(1, 2, 0, 3).reshape(Q, NCORES * CAND)
    scol_q = scol.transpose(1, 2, 0, 3).reshape(Q, NCORES * CAND)
    bad = (scol_q >= NSHARD) | ((allg >= start_i) & (allg < end_i))
    allv[bad] = -np.inf

    # select top-K_HOST by approximate fp8 value, exact-rescore in fp32
    sel = np.argpartition(-allv, K_HOST, axis=1)[:, :K_HOST]
    cols = np.take_along_axis(allg, sel, axis=1)            # [Q, K_HOST]
    qn = q / np.maximum(np.sum(np.abs(q), axis=1, keepdims=True),
                        np.float32(1e-12))
    embg = emb[:, cols.ravel()].reshape(D, Q, K_HOST)
    exact = np.einsum('qd,dqk->qk', qn, embg, optimize=True)
    exact[np.isneginf(np.take_along_axis(allv, sel, axis=1))] = -np.inf

    # top-4 by value desc, index asc on ties (jax.lax.top_k tie rule)
    order = np.lexsort((cols, -exact), axis=1)[:, :TOPK]
    top_v = np.take_along_axis(exact, order, axis=1).astype(np.float32)
    top_i = np.take_along_axis(cols, order, axis=1).astype(np.int32)
    return top_v, top_i


# revision 29
# speedup vs baseline: 1.0023x; 1.0023x over previous
"""Distributed KNN retrieval (Database topk=4) on 8 Trainium2 NeuronCores.

Pipeline (per core, SPMD over 8 cores; corpus sharded along N):
  1. Scan of the core's 50000-column shard in 2048-column chunks:
     fp8e4 DoubleRow matmul (raw queries -- per-query ranking is scale
     invariant) -> PSUM fp32 sims.
  2. ScalarE packs value+index in ONE pass: it copies PSUM fp32 -> bf16
     with a stride-2 write into the HIGH halfwords of a u32 tile whose low
     halfwords are pre-filled with the column iota. The u32 tile read as
     fp32 orders by similarity (low 16 bits are sub-ulp noise) and its low
     bits carry the in-chunk column.
  3. DVE max8 per chunk on packed -> per-core candidate list [128, 200].
     The raw packed lists are DMA'd straight to DRAM -- no device-side
     level-2 / gather / rescore tail at all.
Host merges 8 cores x 200 packed candidates per query, decodes
(value | in-chunk column) + chunk position, exact-rescores the top
candidates in fp32 and takes the global top-4.

The masked range [start, end) is zeroed in the fp8 shard: masked sims are
exactly 0 and never reach the per-core per-chunk top-8 (top sims are
strictly positive); the host additionally filters any candidate falling in
[start, end) or in the shard tail padding before rescoring.
"""

import os

import numpy as np
import ml_dtypes

import concourse.bass as bass
import concourse.bacc as bacc
import concourse.mybir as mybir
import concourse.tile as tile
import concourse.bass_utils as bass_utils

Q, D, N, TOPK = 256, 768, 400000, 4
NCORES = 8
NSHARD = N // NCORES          # 50000
CHUNK = 2048
NCH = 25                      # 24 full chunks + one 848-wide tail chunk
LAST_W = NSHARD - (NCH - 1) * CHUNK   # 848 -- covers the shard exactly
                              # (narrow tail chunk shortens the end-of-scan
                              # pack+max8 critical path; no padding columns).
                              # NOTE: matmul PSUM outputs must stay 512-col
                              # bank-aligned -- segments start at h*512.
KT2 = D // 256                # 3 DoubleRow k-tiles (256-deep each)
MT = Q // 128                 # 2 m-tiles
CAND = NCH * 8                # 200 level-1 candidates per core per query
NPK = 4                       # packed-tile ring depth
EMB_SCALE = 512.0             # fp8 range scaling (ranking invariant)
K_HOST = 64                   # candidates exact-rescored per query on host



_prog_cache = {}


def _install_ntff_hook_shim():
    """Provide antenv.axon_hooks (absent in this image) so that
    run_bass_kernel_spmd(trace=True) can capture NTFF profiles through the
    injected libaxon_pjrt.so. Mirrors trn_agent_boot/trn_boot.py."""
    import sys
    import types
    import ctypes
    import contextlib

    if "antenv.axon_hooks" in sys.modules:
        return
    mod = types.ModuleType("antenv.axon_hooks")
    state = {"hook": None}
    mod.set_axon_ntff_profile_hook = lambda h: state.__setitem__("hook", h)
    mod.get_axon_ntff_profile_hook = lambda: state["hook"]
    sys.modules["antenv.axon_hooks"] = mod

    so_path = "/opt/axon/libaxon_pjrt.so"
    if not os.path.exists(so_path):
        return
    try:
        lib = ctypes.CDLL(so_path)
    except OSError:
        return
    if not hasattr(lib, "axon_start_nrt_profile"):
        return
    lib.axon_start_nrt_profile.argtypes = [ctypes.POINTER(ctypes.c_int64),
                                           ctypes.c_size_t]
    lib.axon_start_nrt_profile.restype = ctypes.c_int64
    lib.axon_stop_nrt_profile.argtypes = [ctypes.c_char_p]
    lib.axon_stop_nrt_profile.restype = ctypes.c_int64

    @contextlib.contextmanager
    def _hook(output_dir, device_ids):
        import jax
        jax.devices()
        if device_ids:
            ids = (ctypes.c_int64 * len(device_ids))(*device_ids)
            rc = lib.axon_start_nrt_profile(ids, len(device_ids))
        else:
            rc = lib.axon_start_nrt_profile(None, 0)
        if rc != 0:
            raise RuntimeError(f"axon_start_nrt_profile rc={rc}")
        try:
            yield
        finally:
            n = lib.axon_stop_nrt_profile(str(output_dir).encode())
            print(f"ntff profile: {n} file(s) written to {output_dir}")

    mod.set_axon_ntff_profile_hook(_hook)


def _build_program():
    nc = bacc.Bacc(None, target_bir_lowering=False, debug=False)
    f8 = mybir.dt.float8e4
    u32 = mybir.dt.uint32
    f32 = mybir.dt.float32
    bf16 = mybir.dt.bfloat16

    # raw queries, fp8, host-laid-out to match the SBUF tile exactly:
    # qt[p, t, i, m] = q8[m, t*256 + i*128 + p] -- one contiguous DMA
    qt_dram = nc.dram_tensor("qT", [128, KT2, 2, Q], f8, kind="ExternalInput")
    # emb shard, fp8, host-packed DoubleRow layout:
    # embL[j, p, (t*2 + i)*CHUNK + n] = emb8[t*256 + i*128 + p, j*CHUNK + n]
    embL = nc.dram_tensor("embL", [NCH, 128, KT2 * 2 * CHUNK], f8,
                          kind="ExternalInput")

    # packed candidate lists, one row of CAND u32 per query (emitted as f32
    # bit patterns; host reinterprets)
    out_cand = nc.dram_tensor("out_cand", [MT, 128, CAND], f32,
                              kind="ExternalOutput")

    DR = mybir.MatmulPerfMode.DoubleRow

    with tile.TileContext(nc) as tc:
        with tc.tile_pool(name="persist", bufs=1) as pp:
            qT = pp.tile([128, KT2, 2, Q], f8, tag="qT")
            vals_all = [pp.tile([128, CAND], f32, tag=f"va{m}", name=f"va{m}")
                        for m in range(MT)]
            # packed ring: low halfwords hold the column iota permanently
            pk = [pp.tile([128, CHUNK], u32, tag=f"pk{i}", name=f"pk{i}")
                  for i in range(NPK)]

            nc.sync.dma_start(qT[:], qt_dram.ap())
            for i in range(NPK):
                nc.gpsimd.iota(pk[i][:], pattern=[[1, CHUNK]], base=0,
                               channel_multiplier=0)

            # ---------- scan shard ----------
            with (
                tc.tile_pool(name="rhs_sb", bufs=6) as rp,
                tc.tile_pool(name="sim_ps", bufs=2, space="PSUM") as sps,
            ):
                # warm the tensor engine to full p-state while the first
                # rhs chunk is still in flight (depends only on qT)
                warm = sps.tile([128, CHUNK], f32, tag="sim")
                for _ in range(6):
                    nc.tensor.matmul(warm[:, :Q], qT[:, 0, :, :128],
                                     qT[:, 0, :, :], start=True, stop=True,
                                     perf_mode=DR)
                for j in range(NCH):
                    w = LAST_W if j == NCH - 1 else CHUNK
                    rhs = rp.tile([128, KT2, 2, CHUNK], f8, tag="rhs")
                    src = embL.ap()[j, :, :KT2 * 2 * w].rearrange(
                        "p (t i n) -> p t i n", t=KT2, i=2)
                    if j == 0:
                        # split per k-tile so the first matmul can start
                        # after one third of the chunk has landed
                        for t in range(KT2):
                            nc.sync.dma_start(rhs[:, t, :, :w], src[:, t])
                    else:
                        # one DMA per chunk: a single large transfer keeps
                        # the 16 DMA engines at full rate and only costs one
                        # serialized SP issue per buffer-release
                        nc.sync.dma_start(rhs[:, :, :, :w], src)
                    for m in range(MT):
                        psum = sps.tile([128, CHUNK], f32, tag="sim")
                        for t in range(KT2):
                            for h0 in range(0, w, 512):
                                h1 = min(w, h0 + 512)
                                nc.tensor.matmul(
                                    psum[:, h0:h1],
                                    qT[:, t, :, m * 128:(m + 1) * 128],
                                    rhs[:, t, :, h0:h1],
                                    start=(t == 0), stop=(t == KT2 - 1),
                                    perf_mode=DR)
                        packed = pk[(j * MT + m) % NPK]
                        dst = packed[:].bitcast(bf16)
                        nc.scalar.copy(dst[:, 1:2 * w:2], psum[:, :w])
                        nc.vector.max(vals_all[m][:, j * 8:(j + 1) * 8],
                                      packed[:, :w].bitcast(f32))
                        if j == NCH - 1:
                            nc.sync.dma_start(out_cand.ap()[m], vals_all[m][:])

    nc.compile()
    return nc


def _get_program():
    if "nc" not in _prog_cache:
        _prog_cache["nc"] = _build_program()
    return _prog_cache["nc"]


def _prepare_core_inputs(q, emb, start, end):
    """Shard + pack fp8 inputs for each core. Returns list of per-core dicts."""
    f8 = ml_dtypes.float8_e4m3
    embs = emb * np.float32(EMB_SCALE)
    if end > start:
        embs[:, start:end] = 0
    emb8 = np.clip(embs, -240, 240).astype(f8)
    q8 = np.clip(np.asarray(q, dtype=np.float32), -240, 240).astype(f8)
    # qt[p, t, i, m] = q8[m, t*256 + i*128 + p]
    qt = np.ascontiguousarray(
        q8.T.reshape(KT2, 2, 128, Q).transpose(2, 0, 1, 3))
    nfull = (NCH - 1) * CHUNK
    in_maps = []
    for c in range(NCORES):
        lo = c * NSHARD
        # embL[j, p, (t*2+i)*w + n] = emb8[t*256 + i*128 + p, lo + j*CHUNK + n]
        embL = np.zeros((NCH, 128, KT2 * 2 * CHUNK), dtype=f8)
        embL[:NCH - 1] = (
            emb8[:, lo:lo + nfull]
            .reshape(KT2 * 2, 128, NCH - 1, CHUNK).transpose(2, 1, 0, 3)
            .reshape(NCH - 1, 128, KT2 * 2 * CHUNK))
        embL[NCH - 1, :, :KT2 * 2 * LAST_W] = (
            emb8[:, lo + nfull:lo + NSHARD]
            .reshape(KT2 * 2, 128, LAST_W).transpose(1, 0, 2)
            .reshape(128, KT2 * 2 * LAST_W))
        in_maps.append({"qT": qt, "embL": embL})
    return in_maps


def kernel(query, embeddings, start, end):
    q = np.asarray(query, dtype=np.float32)
    emb = np.asarray(embeddings, dtype=np.float32)
    start_i = int(np.asarray(start))
    end_i = int(np.asarray(end))
    assert q.shape == (Q, D) and emb.shape == (D, N)

    nc = _get_program()
    in_maps = _prepare_core_inputs(q, emb, start_i, end_i)

    trace = os.environ.get("KNN_TRACE", "0") == "1"
    if trace:
        _install_ntff_hook_shim()
    res = bass_utils.run_bass_kernel_spmd(
        nc, in_maps, core_ids=list(range(NCORES)), trace=trace)
    if trace:
        _prog_cache["last_exec_time_ns"] = res.exec_time_ns
        _prog_cache["last_results"] = res

    # [8, MT, 128, CAND] u32 packed (bf16 sim | in-chunk column)
    packed = np.stack([r["out_cand"] for r in res.results]).view(np.uint32)

    # decode: approximate value from the high halfword, column from the
    # candidate's list position (chunk) + low bits (in-chunk column)
    vals = (packed & np.uint32(0xFFFF0000)).view(np.float32)
    chunk = (np.arange(CAND, dtype=np.int64) >> 3) * CHUNK
    scol = chunk[None, None, None, :] + (packed & np.uint32(2047))
    gcol = scol + (np.arange(NCORES, dtype=np.int64) * NSHARD)[:, None, None, None]

    # per query: [Q, NCORES*CAND]
    allv = vals.transpose(1, 2, 0, 3).reshape(Q, NCORES * CAND).copy()
    allg = gcol.transpose(1, 2, 0, 3).reshape(Q, NCORES * CAND)
    scol_q = scol.transpose(1, 2, 0, 3).reshape(Q, NCORES * CAND)
    bad = (scol_q >= NSHARD) | ((allg >= start_i) & (allg < end_i))
    allv[bad] = -np.inf

    # select top-K_HOST by approximate fp8 value, exact-rescore in fp32
    sel = np.argpartition(-allv, K_HOST, axis=1)[:, :K_HOST]
    cols = np.take_along_axis(allg, sel, axis=1)            # [Q, K_HOST]
    qn = q / np.maximum(np.sum(np.abs(q), axis=1, keepdims=True),
                        np.float32(1e-12))
    embg = emb[:, cols.ravel()].reshape(D, Q, K_HOST)
    exact = np.einsum('qd,dqk->qk', qn, embg, optimize=True)
    exact[np.isneginf(np.take_along_axis(allv, sel, axis=1))] = -np.inf

    # top-4 by value desc, index asc on ties (jax.lax.top_k tie rule)
    order = np.lexsort((cols, -exact), axis=1)[:, :TOPK]
    top_v = np.take_along_axis(exact, order, axis=1).astype(np.float32)
    top_i = np.take_along_axis(cols, order, axis=1).astype(np.int32)
    return top_v, top_i
